# revision 44
# baseline (speedup 1.0000x reference)
"""Trainium2 Bass kernel for nn_Dim6RotStructureHead.

Per token t (B*L = 65536 tokens, data-parallel over 8 cores):
  h  = x @ W1 + b1 ; g = gelu(h)
  LN stats: mu = mean(g), var = E[g^2] - mu^2
  p  = ((g - mu) * rstd * ln_w + ln_b) @ Wp + bp
     = rstd * (g @ Wpa - mu * cs) + k        (Wpa = diag(ln_w) Wp, cs = colsum(Wpa),
                                              k = ln_b @ Wp + bp)
  then Gram-Schmidt rotation + rigid compose + 3-atom placement.

Device layout: hT (d on partitions).  mm1: psum[j,tok] = W1[k,j].T @ xT[k,tok].
q = g @ [Wpa | 1] accumulates into PSUM rows 0..9; sum(g^2) runs concurrently on
PE column-group 1 (tile_position auto from psum base partition 32).
Per-token tail math is done as [128, c, 64] "plane" ops (token = p*64 + T).
"""

import contextlib

import ml_dtypes
import numpy as np

import concourse.bass as bass
import concourse.bacc as bacc
import concourse.mybir as mybir
import concourse.tile as tile
from concourse.bass_utils import run_bass_kernel_spmd

f32 = mybir.dt.float32
f32r = mybir.dt.float32r
bf16 = mybir.dt.bfloat16
AF = mybir.ActivationFunctionType
OP = mybir.AluOpType
AX = mybir.AxisListType

B, L, D = 16, 4096, 512
NCORES = 8
TPC = B * L // NCORES          # 8192 tokens per core
P = 128
NT = TPC // P                  # 64 plane columns
GT = 512                       # tokens per main-loop group
NG = TPC // GT                 # 16 groups
KC = D // P                    # 4 contraction chunks
JC = D // P                    # 4 output chunks
LN_EPS = 1e-5
TRANS_SCALE = 10.0

_NC_CACHE: dict[tuple, object] = {}
_ONES_HOST = np.ones((128, 1), np.float32)
LAST_RESULTS = None


def _bcast3(ap, c):
    """[128, 64] -> [128, c, 64] step-0 broadcast."""
    return ap[:, None, :].to_broadcast((P, c, NT))


def _part_bcast(ap):
    """1-D DRAM AP -> [128, n] partition-broadcast AP (step-0 partition dim)."""
    return bass.AP(tensor=ap.tensor, offset=ap.offset, ap=[[0, P]] + list(ap.ap))


def _build(use_mask: bool, use_k: bool):
    nc = bacc.Bacc()

    xt = nc.declare_dram_parameter("xt", [D, TPC], f32r, isOutput=False)
    aff_in = nc.declare_dram_parameter("aff_in", [P, NT, 12], f32, isOutput=False)
    w1 = nc.declare_dram_parameter("w1", [D, D], f32r, isOutput=False)
    wpa = nc.declare_dram_parameter("wpa", [D, 10], f32r, isOutput=False)
    b1v = nc.declare_dram_parameter("b1v", [D], f32, isOutput=False)
    ncs = nc.declare_dram_parameter("ncs", [9], f32, isOutput=False)
    z1 = nc.declare_dram_parameter("z1", [P, 1], f32r, isOutput=False)
    if use_k:
        kv = nc.declare_dram_parameter("kv", [9], f32, isOutput=False)
        kv10 = nc.declare_dram_parameter("kv10", [3], f32, isOutput=False)
    if use_mask:
        maskf = nc.declare_dram_parameter("maskf", [P, NT], f32, isOutput=False)

    aff_o = nc.declare_dram_parameter("aff_o", [P, 12, NT], f32, isOutput=True)
    xyz_o = nc.declare_dram_parameter("xyz_o", [P, 9, NT], f32, isOutput=True)

    v = nc.vector
    s = nc.scalar
    g = nc.gpsimd

    with tile.TileContext(nc) as tc, contextlib.ExitStack() as ctx:
        const = ctx.enter_context(tc.tile_pool(name="const", bufs=1))
        xpool = ctx.enter_context(tc.tile_pool(name="xpool", bufs=4))
        gpool = ctx.enter_context(tc.tile_pool(name="gpool", bufs=3))
        rows = ctx.enter_context(tc.tile_pool(name="rows", bufs=1))
        planes = ctx.enter_context(tc.tile_pool(name="planes", bufs=1))
        hpsum = ctx.enter_context(tc.tile_pool(name="hpsum", bufs=4, space="PSUM"))
        qpsum = ctx.enter_context(tc.tile_pool(name="qpsum", bufs=2, space="PSUM"))
        spsum = ctx.enter_context(tc.tile_pool(name="spsum", bufs=2, space="PSUM"))
        dpool = ctx.enter_context(tc.tile_pool(name="dpool", bufs=1, space="DRAM"))

        # ---- constants ----
        W1_sb = const.tile([P, KC, D], f32r)
        w1_r = w1.rearrange("(kc kp) j -> kp kc j", kp=P)
        for k in range(KC):
            nc.scalar.dma_start(out=W1_sb[:, k, :], in_=w1_r[:, k, :])
        Wpa_sb = const.tile([P, KC, 10], f32r)
        nc.scalar.dma_start(out=Wpa_sb[:], in_=wpa.rearrange("(kc kp) c -> kp kc c", kp=P))
        Z1_sb = const.tile([P, 1], f32r)
        nc.scalar.dma_start(out=Z1_sb[:], in_=z1[:])
        b1_sb = const.tile([P, JC], f32)
        nc.scalar.dma_start(out=b1_sb[:], in_=b1v.rearrange("(jc jp) -> jp jc", jp=P))
        ncs_sb = const.tile([P, 9], f32)
        nc.scalar.dma_start(out=ncs_sb[:], in_=_part_bcast(ncs[:]))
        NCS9_t = const.tile([P, 9, NT], f32)
        v.tensor_copy(NCS9_t[:], ncs_sb[:, :, None].to_broadcast((P, 9, NT)))
        if use_k:
            kv_sb = const.tile([P, 9], f32)
            nc.sync.dma_start(out=kv_sb[:], in_=_part_bcast(kv[:]))
            kv10_sb = const.tile([P, 3], f32)
            nc.sync.dma_start(out=kv10_sb[:], in_=_part_bcast(kv10[:]))
        A_t = const.tile([P, NT, 12], f32)
        nc.scalar.dma_start(out=A_t[:], in_=aff_in[:])
        if use_mask:
            M_t = const.tile([P, NT], f32)
            nc.sync.dma_start(out=M_t[:], in_=maskf[:])
        eps_sb = const.tile([P, 1], f32)
        v.memset(eps_sb[:], LN_EPS)

        PT_rows = rows.tile([11, TPC], f32)
        S2_row = rows.tile([1, TPC], f32)
        P9 = planes.tile([P, 11, NT], f32)
        ptmp = dpool.tile([11, TPC], f32)
        xt_r = xt.rearrange("(kc kp) t -> kp kc t", kp=P)

        # ---- main loop ----
        pending_s2 = None
        for gi in range(NG):
            t0 = gi * GT
            xt_sb = xpool.tile([P, KC, GT], f32r, tag="xt", name="xt_sb")
            if gi == 0:
                for k in range(KC):
                    nc.sync.dma_start(out=xt_sb[:, k, :],
                                      in_=xt_r[:, k, t0:t0 + GT])
            else:
                nc.sync.dma_start(out=xt_sb[:, 0:2, :],
                                  in_=xt_r[:, 0:2, t0:t0 + GT])
                nc.sync.dma_start(out=xt_sb[:, 2:4, :],
                                  in_=xt_r[:, 2:4, t0:t0 + GT])

            gts = []
            g2s = []
            for j in range(JC):
                ph = hpsum.tile([P, GT], f32, tag="hps", name="ph")
                for k in range(KC):
                    nc.tensor.matmul(
                        ph[:],
                        lhsT=W1_sb[:, k, j * P:(j + 1) * P],
                        rhs=xt_sb[:, k, :],
                        start=(k == 0),
                        stop=(k == KC - 1),
                    )
                gt = gpool.tile([P, GT], f32r, tag=f"gt{j}", name="gt")
                s.activation(gt[:], ph[:], AF.Gelu, bias=b1_sb[:, j:j + 1], scale=1.0)
                g2 = gpool.tile([P, GT], bf16, tag=f"g2{j}", name="g2")
                if j % 2 == 0:
                    s.activation(g2[:], gt[:], AF.Square)
                else:
                    v.tensor_tensor(g2[:], gt[:], gt[:], OP.mult)
                gts.append(gt)
                g2s.append(g2)

            # reduce the four g^2 chunks on DVE/GPSIMD; a single deferred
            # matmul (emitted under the NEXT group's mm1 shadow) does the
            # final partition-sum, saving 3 full PE token-streams per group.
            a01 = gpool.tile([P, GT], f32, tag="a01", name="a01")
            v.tensor_tensor(a01[:], g2s[0][:], g2s[1][:], OP.add)
            a23 = gpool.tile([P, GT], f32, tag="a23", name="a23")
            v.tensor_tensor(a23[:], g2s[2][:], g2s[3][:], OP.add)
            g2sum = gpool.tile([P, GT], f32r, tag="g2sum", name="g2sum")
            v.tensor_tensor(g2sum[:], a01[:], a23[:], OP.add)
            if pending_s2 is not None:
                prev_t0, prev_sum = pending_s2
                ps2 = spsum.tile([1, GT], f32, tag="sps", name="ps2")
                nc.tensor.matmul(ps2[:], lhsT=Z1_sb[:], rhs=prev_sum[:],
                                 start=True, stop=True)
                s.copy(S2_row[0:1, prev_t0:prev_t0 + GT], ps2[:])
                nc.gpsimd.dma_start(out=ptmp[10:11, prev_t0:prev_t0 + GT],
                                    in_=S2_row[0:1, prev_t0:prev_t0 + GT])
            pending_s2 = (t0, g2sum)
            pq = qpsum.tile([10, GT], f32, tag="qps", name="pq")
            for k in range(KC):
                nc.tensor.matmul(pq[:], lhsT=Wpa_sb[:, k, :], rhs=gts[k][:],
                                 start=(k == 0), stop=(k == KC - 1))
            s.copy(PT_rows[0:10, t0:t0 + GT], pq[:])

            # incremental planes import: this group's 512 tokens are
            # partitions 8*gi .. 8*gi+7 of the plane layout (token = p*64+T)
            nc.gpsimd.dma_start(out=ptmp[0:10, t0:t0 + GT],
                                in_=PT_rows[0:10, t0:t0 + GT])
            nc.gpsimd.dma_start(out=ptmp[10:11, t0:t0 + GT],
                                in_=S2_row[0:1, t0:t0 + GT])
            nc.gpsimd.dma_start(
                out=P9[8 * gi:8 * gi + 8, 0:10, :],
                in_=ptmp[0:10, t0:t0 + GT].rearrange("c (p t) -> p c t", p=8))
            if gi >= 1:
                p0 = (gi - 1) * GT
                nc.gpsimd.dma_start(
                    out=P9[8 * (gi - 1):8 * (gi - 1) + 8, 10:11, :],
                    in_=ptmp[10:11, p0:p0 + GT].rearrange("c (p t) -> p c t", p=8))

        prev_t0, prev_sum = pending_s2
        ps2f = spsum.tile([1, GT], f32, tag="sps", name="ps2f")
        nc.tensor.matmul(ps2f[:], lhsT=Z1_sb[:], rhs=prev_sum[:],
                         start=True, stop=True)
        s.copy(S2_row[0:1, prev_t0:prev_t0 + GT], ps2f[:])
        nc.sync.dma_start(out=ptmp[10:11, prev_t0:prev_t0 + GT],
                          in_=S2_row[0:1, prev_t0:prev_t0 + GT])
        nc.sync.dma_start(
            out=P9[8 * (NG - 1):, 10:11, :],
            in_=ptmp[10:11, prev_t0:prev_t0 + GT].rearrange(
                "c (p t) -> p c t", p=8))

        def pt(shape, name):
            return planes.tile(shape, f32, tag=name, name=name)

        S1 = P9[:, 9, :]
        S2 = P9[:, 10, :]
        mu = pt([P, NT], "mu")
        v.tensor_scalar_mul(mu[:], S1, 1.0 / D)
        mu2 = pt([P, NT], "mu2")
        v.tensor_tensor(mu2[:], mu[:], mu[:], OP.mult)
        e2t = pt([P, NT], "e2t")
        v.tensor_scalar_mul(e2t[:], S2, 1.0 / D)
        varr = pt([P, NT], "varr")
        v.scalar_tensor_tensor(varr[:], mu2[:], -1.0, e2t[:], OP.mult, OP.add)
        A_ = pt([P, NT], "A_")
        s.activation(A_[:], varr[:], AF.Sqrt, bias=eps_sb[:, 0:1])
        rstd = pt([P, NT], "rstd")
        v.reciprocal(rstd[:], A_[:])
        rstd10 = pt([P, NT], "rstd10")
        v.tensor_scalar_mul(rstd10[:], rstd[:], TRANS_SCALE)

        # U_c = q_c - mu*cs_c   (ncs holds -cs)
        U = planes.tile([P, 9, NT], f32)
        MC = pt([P, 9, NT], "MC")
        v.tensor_tensor(MC[:], NCS9_t[:], _bcast3(mu, 9), OP.mult)
        v.tensor_tensor(U[:], P9[:, 0:9, :], MC[:], OP.add)

        TR = pt([P, 3, NT], "TR")
        v.tensor_tensor(TR[:], U[:, 0:3, :], _bcast3(rstd10, 3), OP.mult)
        P36 = pt([P, 6, NT], "P36")
        v.tensor_tensor(P36[:], U[:, 3:9, :], _bcast3(rstd, 6), OP.mult)
        if use_k:
            for c in range(3):
                v.tensor_scalar_add(TR[:, c, :], TR[:, c, :], kv10_sb[:, c:c + 1])
            for c in range(6):
                v.tensor_scalar_add(P36[:, c, :], P36[:, c, :], kv_sb[:, 3 + c:4 + c])

        SQ6 = pt([P, 6, NT], "SQ6")
        v.tensor_tensor(SQ6[:], P36[:], P36[:], OP.mult)
        M2 = pt([P, 2, NT], "M2")
        v.tensor_reduce(M2[:, 0, :], SQ6[:, 0:3, :].rearrange("p c t -> p t c"),
                        axis=AX.X, op=OP.add)
        v.tensor_reduce(M2[:, 1, :], SQ6[:, 3:6, :].rearrange("p c t -> p t c"),
                        axis=AX.X, op=OP.add)
        NXY = pt([P, 2, NT], "NXY")
        s.activation(NXY[:], M2[:], AF.Sqrt)

        DEN = pt([P, 2, NT], "DEN")
        v.tensor_scalar_add(DEN[:], NXY[:], 1e-5)
        CXY = pt([P, 2, NT], "CXY")
        v.reciprocal(CXY[:], DEN[:])
        cx = CXY[:, 0, :]
        cy = CXY[:, 1, :]

        VX = pt([P, 3, NT], "VX")
        v.tensor_tensor(VX[:], P36[:, 0:3, :], _bcast3(CXY[:, 0, :], 3), OP.mult)
        VY = pt([P, 3, NT], "VY")
        v.tensor_tensor(VY[:], P36[:, 3:6, :], _bcast3(CXY[:, 1, :], 3), OP.mult)

        # e1 = -vec_x (unit to ~1e-6; reference's renorm is absmax-negligible)
        EALL = planes.tile([P, 3, 3, NT], f32)
        v.tensor_scalar_mul(EALL[:, 0, :, :], VX[:], -1.0)
        XY = VY

        DP = pt([P, NT, 3], "DP")
        v.tensor_tensor(DP[:], XY[:].rearrange("p c t -> p t c"),
                        EALL[:, 0, :, :].rearrange("p c t -> p t c"), OP.mult)
        dd = pt([P, NT], "dd")
        v.tensor_reduce(dd[:], DP[:], axis=AX.X, op=OP.add)
        T6 = pt([P, 3, NT], "T6")
        v.tensor_tensor(T6[:], EALL[:, 0, :, :], _bcast3(dd, 3), OP.mult)
        U2 = pt([P, 3, NT], "U2")
        v.tensor_tensor(U2[:], XY[:], T6[:], OP.subtract)

        SQ3 = pt([P, 3, NT], "SQ3")
        v.tensor_tensor(SQ3[:], U2[:], U2[:], OP.mult)
        m2u = pt([P, NT], "m2u")
        v.tensor_reduce(m2u[:], SQ3[:].rearrange("p c t -> p t c"),
                        axis=AX.X, op=OP.add)
        su = pt([P, NT], "su")
        s.activation(su[:], m2u[:], AF.Sqrt)
        v.tensor_scalar_add(su[:], su[:], 1e-10)
        r3 = pt([P, NT], "r3")
        v.reciprocal(r3[:], su[:])
        v.tensor_tensor(EALL[:, 1, :, :], U2[:], _bcast3(r3, 3), OP.mult)

        # e3 = e1 x e2  (gpsimd)
        ca = pt([P, NT], "ca")
        cb = pt([P, NT], "cb")
        E1 = EALL[:, 0, :, :]
        E2 = EALL[:, 1, :, :]
        for i in range(3):
            i1, i2 = (i + 1) % 3, (i + 2) % 3
            v.tensor_tensor(ca[:], E1[:, i1, :], E2[:, i2, :], OP.mult)
            v.tensor_tensor(cb[:], E1[:, i2, :], E2[:, i1, :], OP.mult)
            v.tensor_tensor(EALL[:, 2, i, :], ca[:], cb[:], OP.subtract)

        # mask: Ru -> m*(Ru - I) + I ; tu -> m*trans
        if use_mask:
            mtmp = pt([P, NT], "mtmp")
            for j in range(3):
                for k in range(3):
                    if j == k:
                        v.tensor_scalar_add(mtmp[:], EALL[:, j, k, :], -1.0)
                        v.tensor_tensor(mtmp[:], mtmp[:], M_t[:], OP.mult)
                        v.tensor_scalar_add(EALL[:, j, k, :], mtmp[:], 1.0)
                    else:
                        v.tensor_tensor(EALL[:, j, k, :], EALL[:, j, k, :],
                                        M_t[:], OP.mult)
            TU = pt([P, 3, NT], "TU")
            g.tensor_tensor(TU[:], TR[:], _bcast3(M_t[:], 3), OP.mult)
        else:
            TU = TR

        # R[i][j] = dot(Rp_row_i, e_j) ; t = Rp @ tu + tp
        RT = planes.tile([P, 12, NT], f32)
        for i in range(3):
            for j in range(3):
                DPR = pt([P, NT, 3], f"DPR{(i + j) % 2}")
                v.tensor_tensor(DPR[:], A_t[:, :, 3 * i:3 * i + 3],
                                EALL[:, j, :, :].rearrange("p k t -> p t k"),
                                OP.mult)
                v.tensor_reduce(RT[:, 3 * i + j, :], DPR[:], axis=AX.X, op=OP.add)
        for i in range(3):
            DPT = pt([P, NT, 3], f"DPT{i % 2}")
            v.tensor_tensor(DPT[:], A_t[:, :, 3 * i:3 * i + 3],
                            TU[:].rearrange("p k t -> p t k"), OP.mult)
            tacc = pt([P, NT], f"tacc{i}")
            v.tensor_reduce(tacc[:], DPT[:], axis=AX.X, op=OP.add)
            v.tensor_tensor(RT[:, 9 + i, :], tacc[:], A_t[:, :, 9 + i], OP.add)

        # pred_xyz
        XYZ = planes.tile([P, 9, NT], f32)
        for i in range(3):
            v.scalar_tensor_tensor(XYZ[:, i, :], RT[:, 3 * i, :], 0.5256,
                                   RT[:, 9 + i, :], OP.mult, OP.add)
            v.scalar_tensor_tensor(XYZ[:, i, :], RT[:, 3 * i + 1, :], 1.3612,
                                   XYZ[:, i, :], OP.mult, OP.add)
            g.tensor_copy(XYZ[:, 3 + i, :], RT[:, 9 + i, :])
            v.scalar_tensor_tensor(XYZ[:, 6 + i, :], RT[:, 3 * i, :], -1.5251,
                                   RT[:, 9 + i, :], OP.mult, OP.add)

        nc.gpsimd.dma_start(out=aff_o[:, 0:9, :], in_=RT[:, 0:9, :])
        nc.scalar.dma_start(out=aff_o[:, 9:12, :], in_=RT[:, 9:12, :])
        nc.gpsimd.dma_start(out=xyz_o[0:64], in_=XYZ[0:64])
        nc.scalar.dma_start(out=xyz_o[64:128], in_=XYZ[64:128])

    nc.finalize()
    return nc


def _get_nc(use_mask: bool, use_k: bool):
    key = (use_mask, use_k)
    if key not in _NC_CACHE:
        _NC_CACHE[key] = _build(use_mask, use_k)
    return _NC_CACHE[key]


def kernel(x, affine, affine_mask, W1, b1, ln_w, ln_b, Wp, bp):
    global LAST_RESULTS
    x = np.ascontiguousarray(np.asarray(x, dtype=np.float32))
    affine = np.ascontiguousarray(np.asarray(affine, dtype=np.float32))
    mask = np.asarray(affine_mask)
    W1 = np.ascontiguousarray(np.asarray(W1, dtype=np.float32))
    b1 = np.ascontiguousarray(np.asarray(b1, dtype=np.float32))
    ln_w = np.asarray(ln_w, dtype=np.float32)
    ln_b = np.asarray(ln_b, dtype=np.float32)
    Wp = np.asarray(Wp, dtype=np.float32)
    bp = np.asarray(bp, dtype=np.float32)

    wpa9 = (ln_w[:, None] * Wp).astype(np.float32)                      # [512, 9]
    wpa = np.concatenate([wpa9, np.ones((D, 1), np.float32)], axis=1)   # [512, 10]
    ncs = (-wpa9.astype(np.float64).sum(0)).astype(np.float32)          # -colsum
    kvec = (ln_b.astype(np.float64) @ Wp.astype(np.float64)
            + bp.astype(np.float64)).astype(np.float32)                 # [9]
    use_k = bool(np.any(kvec != 0.0))
    use_mask = not bool(mask.all())

    nc = _get_nc(use_mask, use_k)

    xs = x.reshape(NCORES, TPC, D)
    affs = affine.reshape(NCORES, P, NT, 12)
    maskfs = mask.reshape(NCORES, P, NT).astype(np.float32)

    in_maps = []
    for c in range(NCORES):
        im = {
            "xt": np.ascontiguousarray(xs[c].T),
            "aff_in": affs[c],
            "w1": W1,
            "wpa": wpa,
            "b1v": b1,
            "ncs": ncs,
            "z1": _ONES_HOST,
        }
        if use_k:
            im["kv"] = kvec
            im["kv10"] = (kvec[0:3] * TRANS_SCALE).astype(np.float32)
        if use_mask:
            im["maskf"] = maskfs[c]
        in_maps.append(im)

    res = run_bass_kernel_spmd(nc, in_maps, list(range(NCORES)))
    LAST_RESULTS = res

    # [128, c, 64] planes -> [8192, c] token-major
    aff_out = np.concatenate(
        [res.results[c]["aff_o"].transpose(0, 2, 1).reshape(TPC, 12)[None]
         for c in range(NCORES)], axis=0,
    ).reshape(B, L, 12).astype(np.float32)
    xyz_out = np.concatenate(
        [res.results[c]["xyz_o"].transpose(0, 2, 1).reshape(TPC, 9)[None]
         for c in range(NCORES)], axis=0,
    ).reshape(B, L, 3, 3).astype(np.float32)
    return aff_out, xyz_out


# revision 45
# speedup vs baseline: 1.0180x; 1.0180x over previous
"""Trainium2 Bass kernel for nn_Dim6RotStructureHead.

Per token t (B*L = 65536 tokens, data-parallel over 8 cores):
  h  = x @ W1 + b1 ; g = gelu(h)
  LN stats: mu = mean(g), var = E[g^2] - mu^2
  p  = ((g - mu) * rstd * ln_w + ln_b) @ Wp + bp
     = rstd * (g @ Wpa - mu * cs) + k        (Wpa = diag(ln_w) Wp, cs = colsum(Wpa),
                                              k = ln_b @ Wp + bp)
  then Gram-Schmidt rotation + rigid compose + 3-atom placement.

Device layout: hT (d on partitions).  mm1: psum[j,tok] = W1[k,j].T @ xT[k,tok].
q = g @ [Wpa | 1] accumulates into PSUM rows 0..9; sum(g^2) runs concurrently on
PE column-group 1 (tile_position auto from psum base partition 32).
Per-token tail math is done as [128, c, 64] "plane" ops (token = p*64 + T).
"""

import contextlib

import ml_dtypes
import numpy as np

import concourse.bass as bass
import concourse.bacc as bacc
import concourse.mybir as mybir
import concourse.tile as tile
from concourse.bass_utils import run_bass_kernel_spmd

f32 = mybir.dt.float32
f32r = mybir.dt.float32r
bf16 = mybir.dt.bfloat16
AF = mybir.ActivationFunctionType
OP = mybir.AluOpType
AX = mybir.AxisListType

B, L, D = 16, 4096, 512
NCORES = 8
TPC = B * L // NCORES          # 8192 tokens per core
P = 128
NT = TPC // P                  # 64 plane columns
GT = 512                       # tokens per main-loop group
NG = TPC // GT                 # 16 groups
KC = D // P                    # 4 contraction chunks
JC = D // P                    # 4 output chunks
LN_EPS = 1e-5
TRANS_SCALE = 10.0

_NC_CACHE: dict[tuple, object] = {}
_ONES_HOST = np.ones((128, 1), np.float32)
LAST_RESULTS = None


def _bcast3(ap, c):
    """[128, 64] -> [128, c, 64] step-0 broadcast."""
    return ap[:, None, :].to_broadcast((P, c, NT))


def _part_bcast(ap):
    """1-D DRAM AP -> [128, n] partition-broadcast AP (step-0 partition dim)."""
    return bass.AP(tensor=ap.tensor, offset=ap.offset, ap=[[0, P]] + list(ap.ap))


def _build(use_mask: bool, use_k: bool):
    nc = bacc.Bacc()

    xt = nc.declare_dram_parameter("xt", [D, TPC], f32r, isOutput=False)
    aff_in = nc.declare_dram_parameter("aff_in", [P, NT, 12], f32, isOutput=False)
    w1 = nc.declare_dram_parameter("w1", [D, D], f32r, isOutput=False)
    wpa = nc.declare_dram_parameter("wpa", [D, 10], f32r, isOutput=False)
    b1v = nc.declare_dram_parameter("b1v", [D], f32, isOutput=False)
    ncs = nc.declare_dram_parameter("ncs", [9], f32, isOutput=False)
    z1 = nc.declare_dram_parameter("z1", [P, 1], f32r, isOutput=False)
    if use_k:
        kv = nc.declare_dram_parameter("kv", [9], f32, isOutput=False)
        kv10 = nc.declare_dram_parameter("kv10", [3], f32, isOutput=False)
    if use_mask:
        maskf = nc.declare_dram_parameter("maskf", [P, NT], f32, isOutput=False)

    aff_o = nc.declare_dram_parameter("aff_o", [P, 12, NT], f32, isOutput=True)
    xyz_o = nc.declare_dram_parameter("xyz_o", [P, 9, NT], f32, isOutput=True)

    v = nc.vector
    s = nc.scalar
    g = nc.gpsimd

    with tile.TileContext(nc) as tc, contextlib.ExitStack() as ctx:
        const = ctx.enter_context(tc.tile_pool(name="const", bufs=1))
        xpool = ctx.enter_context(tc.tile_pool(name="xpool", bufs=4))
        gpool = ctx.enter_context(tc.tile_pool(name="gpool", bufs=3))
        rows = ctx.enter_context(tc.tile_pool(name="rows", bufs=1))
        planes = ctx.enter_context(tc.tile_pool(name="planes", bufs=1))
        hpsum = ctx.enter_context(tc.tile_pool(name="hpsum", bufs=4, space="PSUM"))
        qpsum = ctx.enter_context(tc.tile_pool(name="qpsum", bufs=2, space="PSUM"))
        spsum = ctx.enter_context(tc.tile_pool(name="spsum", bufs=2, space="PSUM"))
        dpool = ctx.enter_context(tc.tile_pool(name="dpool", bufs=1, space="DRAM"))

        # ---- constants ----
        W1_sb = const.tile([P, KC, D], f32r)
        w1_r = w1.rearrange("(kc kp) j -> kp kc j", kp=P)
        for k in range(KC):
            nc.scalar.dma_start(out=W1_sb[:, k, :], in_=w1_r[:, k, :])
        Wpa_sb = const.tile([P, KC, 10], f32r)
        nc.scalar.dma_start(out=Wpa_sb[:], in_=wpa.rearrange("(kc kp) c -> kp kc c", kp=P))
        Z1_sb = const.tile([P, 1], f32r)
        nc.scalar.dma_start(out=Z1_sb[:], in_=z1[:])
        b1_sb = const.tile([P, JC], f32)
        nc.scalar.dma_start(out=b1_sb[:], in_=b1v.rearrange("(jc jp) -> jp jc", jp=P))
        ncs_sb = const.tile([P, 9], f32)
        nc.scalar.dma_start(out=ncs_sb[:], in_=_part_bcast(ncs[:]))
        NCS9_t = const.tile([P, 9, NT], f32)
        v.tensor_copy(NCS9_t[:], ncs_sb[:, :, None].to_broadcast((P, 9, NT)))
        if use_k:
            kv_sb = const.tile([P, 9], f32)
            nc.sync.dma_start(out=kv_sb[:], in_=_part_bcast(kv[:]))
            kv10_sb = const.tile([P, 3], f32)
            nc.sync.dma_start(out=kv10_sb[:], in_=_part_bcast(kv10[:]))
        A_t = const.tile([P, NT, 12], f32)
        nc.scalar.dma_start(out=A_t[:], in_=aff_in[:])
        if use_mask:
            M_t = const.tile([P, NT], f32)
            nc.sync.dma_start(out=M_t[:], in_=maskf[:])
        eps_sb = const.tile([P, 1], f32)
        v.memset(eps_sb[:], LN_EPS)

        PT_rows = rows.tile([11, TPC], f32)
        S2_row = rows.tile([1, TPC], f32)
        P9 = planes.tile([P, 11, NT], f32)
        ptmp = dpool.tile([11, TPC], f32)
        xt_r = xt.rearrange("(kc kp) t -> kp kc t", kp=P)

        # ---- main loop ----
        pending_s2 = None
        for gi in range(NG):
            t0 = gi * GT
            xt_sb = xpool.tile([P, KC, GT], f32r, tag="xt", name="xt_sb")
            if gi == 0:
                for k in range(KC):
                    nc.sync.dma_start(out=xt_sb[:, k, :],
                                      in_=xt_r[:, k, t0:t0 + GT])
            else:
                nc.sync.dma_start(out=xt_sb[:, 0:2, :],
                                  in_=xt_r[:, 0:2, t0:t0 + GT])
                nc.sync.dma_start(out=xt_sb[:, 2:4, :],
                                  in_=xt_r[:, 2:4, t0:t0 + GT])

            gts = []
            g2s = []
            for j in range(JC):
                ph = hpsum.tile([P, GT], f32, tag="hps", name="ph")
                for k in range(KC):
                    nc.tensor.matmul(
                        ph[:],
                        lhsT=W1_sb[:, k, j * P:(j + 1) * P],
                        rhs=xt_sb[:, k, :],
                        start=(k == 0),
                        stop=(k == KC - 1),
                    )
                gt = gpool.tile([P, GT], f32r, tag=f"gt{j}", name="gt")
                s.activation(gt[:], ph[:], AF.Gelu, bias=b1_sb[:, j:j + 1], scale=1.0)
                g2 = gpool.tile([P, GT], bf16, tag=f"g2{j}", name="g2")
                if j % 2 == 0:
                    s.activation(g2[:], gt[:], AF.Square)
                else:
                    v.tensor_tensor(g2[:], gt[:], gt[:], OP.mult)
                gts.append(gt)
                g2s.append(g2)

            # reduce the four g^2 chunks on DVE/GPSIMD; a single deferred
            # matmul (emitted under the NEXT group's mm1 shadow) does the
            # final partition-sum, saving 3 full PE token-streams per group.
            a01 = gpool.tile([P, GT], f32, tag="a01", name="a01")
            v.tensor_tensor(a01[:], g2s[0][:], g2s[1][:], OP.add)
            a23 = gpool.tile([P, GT], f32, tag="a23", name="a23")
            v.tensor_tensor(a23[:], g2s[2][:], g2s[3][:], OP.add)
            g2sum = gpool.tile([P, GT], f32r, tag="g2sum", name="g2sum")
            v.tensor_tensor(g2sum[:], a01[:], a23[:], OP.add)
            if pending_s2 is not None:
                prev_t0, prev_sum = pending_s2
                ps2 = spsum.tile([1, GT], f32, tag="sps", name="ps2")
                nc.tensor.matmul(ps2[:], lhsT=Z1_sb[:], rhs=prev_sum[:],
                                 start=True, stop=True)
                s.copy(S2_row[0:1, prev_t0:prev_t0 + GT], ps2[:])
                nc.gpsimd.dma_start(out=ptmp[10:11, prev_t0:prev_t0 + GT],
                                    in_=S2_row[0:1, prev_t0:prev_t0 + GT])
            pending_s2 = (t0, g2sum)
            pq = qpsum.tile([10, GT], f32, tag="qps", name="pq")
            for k in range(KC):
                nc.tensor.matmul(pq[:], lhsT=Wpa_sb[:, k, :], rhs=gts[k][:],
                                 start=(k == 0), stop=(k == KC - 1))
            v.tensor_copy(PT_rows[0:10, t0:t0 + GT], pq[:])

            # incremental planes import: this group's 512 tokens are
            # partitions 8*gi .. 8*gi+7 of the plane layout (token = p*64+T)
            nc.gpsimd.dma_start(out=ptmp[0:10, t0:t0 + GT],
                                in_=PT_rows[0:10, t0:t0 + GT])
            nc.gpsimd.dma_start(out=ptmp[10:11, t0:t0 + GT],
                                in_=S2_row[0:1, t0:t0 + GT])
            nc.gpsimd.dma_start(
                out=P9[8 * gi:8 * gi + 8, 0:10, :],
                in_=ptmp[0:10, t0:t0 + GT].rearrange("c (p t) -> p c t", p=8))
            if gi >= 1:
                p0 = (gi - 1) * GT
                nc.gpsimd.dma_start(
                    out=P9[8 * (gi - 1):8 * (gi - 1) + 8, 10:11, :],
                    in_=ptmp[10:11, p0:p0 + GT].rearrange("c (p t) -> p c t", p=8))

        prev_t0, prev_sum = pending_s2
        ps2f = spsum.tile([1, GT], f32, tag="sps", name="ps2f")
        nc.tensor.matmul(ps2f[:], lhsT=Z1_sb[:], rhs=prev_sum[:],
                         start=True, stop=True)
        s.copy(S2_row[0:1, prev_t0:prev_t0 + GT], ps2f[:])
        nc.sync.dma_start(out=ptmp[10:11, prev_t0:prev_t0 + GT],
                          in_=S2_row[0:1, prev_t0:prev_t0 + GT])
        nc.sync.dma_start(
            out=P9[8 * (NG - 1):, 10:11, :],
            in_=ptmp[10:11, prev_t0:prev_t0 + GT].rearrange(
                "c (p t) -> p c t", p=8))

        def pt(shape, name):
            return planes.tile(shape, f32, tag=name, name=name)

        S1 = P9[:, 9, :]
        S2 = P9[:, 10, :]
        mu = pt([P, NT], "mu")
        v.tensor_scalar_mul(mu[:], S1, 1.0 / D)
        mu2 = pt([P, NT], "mu2")
        v.tensor_tensor(mu2[:], mu[:], mu[:], OP.mult)
        e2t = pt([P, NT], "e2t")
        v.tensor_scalar_mul(e2t[:], S2, 1.0 / D)
        varr = pt([P, NT], "varr")
        v.scalar_tensor_tensor(varr[:], mu2[:], -1.0, e2t[:], OP.mult, OP.add)
        A_ = pt([P, NT], "A_")
        s.activation(A_[:], varr[:], AF.Sqrt, bias=eps_sb[:, 0:1])
        rstd = pt([P, NT], "rstd")
        v.reciprocal(rstd[:], A_[:])
        rstd10 = pt([P, NT], "rstd10")
        v.tensor_scalar_mul(rstd10[:], rstd[:], TRANS_SCALE)

        # U_c = q_c - mu*cs_c   (ncs holds -cs)
        U = planes.tile([P, 9, NT], f32)
        MC = pt([P, 9, NT], "MC")
        v.tensor_tensor(MC[:], NCS9_t[:], _bcast3(mu, 9), OP.mult)
        v.tensor_tensor(U[:], P9[:, 0:9, :], MC[:], OP.add)

        TR = pt([P, 3, NT], "TR")
        v.tensor_tensor(TR[:], U[:, 0:3, :], _bcast3(rstd10, 3), OP.mult)
        P36 = pt([P, 6, NT], "P36")
        v.tensor_tensor(P36[:], U[:, 3:9, :], _bcast3(rstd, 6), OP.mult)
        if use_k:
            for c in range(3):
                v.tensor_scalar_add(TR[:, c, :], TR[:, c, :], kv10_sb[:, c:c + 1])
            for c in range(6):
                v.tensor_scalar_add(P36[:, c, :], P36[:, c, :], kv_sb[:, 3 + c:4 + c])

        SQ6 = pt([P, 6, NT], "SQ6")
        v.tensor_tensor(SQ6[:], P36[:], P36[:], OP.mult)
        M2 = pt([P, 2, NT], "M2")
        v.tensor_reduce(M2[:, 0, :], SQ6[:, 0:3, :].rearrange("p c t -> p t c"),
                        axis=AX.X, op=OP.add)
        v.tensor_reduce(M2[:, 1, :], SQ6[:, 3:6, :].rearrange("p c t -> p t c"),
                        axis=AX.X, op=OP.add)
        NXY = pt([P, 2, NT], "NXY")
        s.activation(NXY[:], M2[:], AF.Sqrt)

        DEN = pt([P, 2, NT], "DEN")
        v.tensor_scalar_add(DEN[:], NXY[:], 1e-5)
        CXY = pt([P, 2, NT], "CXY")
        v.reciprocal(CXY[:], DEN[:])
        cx = CXY[:, 0, :]
        cy = CXY[:, 1, :]

        VX = pt([P, 3, NT], "VX")
        v.tensor_tensor(VX[:], P36[:, 0:3, :], _bcast3(CXY[:, 0, :], 3), OP.mult)
        VY = pt([P, 3, NT], "VY")
        v.tensor_tensor(VY[:], P36[:, 3:6, :], _bcast3(CXY[:, 1, :], 3), OP.mult)

        # e1 = -vec_x (unit to ~1e-6; reference's renorm is absmax-negligible)
        EALL = planes.tile([P, 3, 3, NT], f32)
        v.tensor_scalar_mul(EALL[:, 0, :, :], VX[:], -1.0)
        XY = VY

        DP = pt([P, NT, 3], "DP")
        v.tensor_tensor(DP[:], XY[:].rearrange("p c t -> p t c"),
                        EALL[:, 0, :, :].rearrange("p c t -> p t c"), OP.mult)
        dd = pt([P, NT], "dd")
        v.tensor_reduce(dd[:], DP[:], axis=AX.X, op=OP.add)
        T6 = pt([P, 3, NT], "T6")
        v.tensor_tensor(T6[:], EALL[:, 0, :, :], _bcast3(dd, 3), OP.mult)
        U2 = pt([P, 3, NT], "U2")
        v.tensor_tensor(U2[:], XY[:], T6[:], OP.subtract)

        SQ3 = pt([P, 3, NT], "SQ3")
        v.tensor_tensor(SQ3[:], U2[:], U2[:], OP.mult)
        m2u = pt([P, NT], "m2u")
        v.tensor_reduce(m2u[:], SQ3[:].rearrange("p c t -> p t c"),
                        axis=AX.X, op=OP.add)
        su = pt([P, NT], "su")
        s.activation(su[:], m2u[:], AF.Sqrt)
        v.tensor_scalar_add(su[:], su[:], 1e-10)
        r3 = pt([P, NT], "r3")
        v.reciprocal(r3[:], su[:])
        v.tensor_tensor(EALL[:, 1, :, :], U2[:], _bcast3(r3, 3), OP.mult)

        # e3 = e1 x e2  (gpsimd)
        ca = pt([P, NT], "ca")
        cb = pt([P, NT], "cb")
        E1 = EALL[:, 0, :, :]
        E2 = EALL[:, 1, :, :]
        for i in range(3):
            i1, i2 = (i + 1) % 3, (i + 2) % 3
            v.tensor_tensor(ca[:], E1[:, i1, :], E2[:, i2, :], OP.mult)
            v.tensor_tensor(cb[:], E1[:, i2, :], E2[:, i1, :], OP.mult)
            v.tensor_tensor(EALL[:, 2, i, :], ca[:], cb[:], OP.subtract)

        # mask: Ru -> m*(Ru - I) + I ; tu -> m*trans
        if use_mask:
            mtmp = pt([P, NT], "mtmp")
            for j in range(3):
                for k in range(3):
                    if j == k:
                        v.tensor_scalar_add(mtmp[:], EALL[:, j, k, :], -1.0)
                        v.tensor_tensor(mtmp[:], mtmp[:], M_t[:], OP.mult)
                        v.tensor_scalar_add(EALL[:, j, k, :], mtmp[:], 1.0)
                    else:
                        v.tensor_tensor(EALL[:, j, k, :], EALL[:, j, k, :],
                                        M_t[:], OP.mult)
            TU = pt([P, 3, NT], "TU")
            g.tensor_tensor(TU[:], TR[:], _bcast3(M_t[:], 3), OP.mult)
        else:
            TU = TR

        # R[i][j] = dot(Rp_row_i, e_j) ; t = Rp @ tu + tp
        RT = planes.tile([P, 12, NT], f32)
        for i in range(3):
            for j in range(3):
                DPR = pt([P, NT, 3], f"DPR{(i + j) % 2}")
                v.tensor_tensor(DPR[:], A_t[:, :, 3 * i:3 * i + 3],
                                EALL[:, j, :, :].rearrange("p k t -> p t k"),
                                OP.mult)
                v.tensor_reduce(RT[:, 3 * i + j, :], DPR[:], axis=AX.X, op=OP.add)
        for i in range(3):
            DPT = pt([P, NT, 3], f"DPT{i % 2}")
            v.tensor_tensor(DPT[:], A_t[:, :, 3 * i:3 * i + 3],
                            TU[:].rearrange("p k t -> p t k"), OP.mult)
            tacc = pt([P, NT], f"tacc{i}")
            v.tensor_reduce(tacc[:], DPT[:], axis=AX.X, op=OP.add)
            v.tensor_tensor(RT[:, 9 + i, :], tacc[:], A_t[:, :, 9 + i], OP.add)

        # pred_xyz
        XYZ = planes.tile([P, 9, NT], f32)
        for i in range(3):
            v.scalar_tensor_tensor(XYZ[:, i, :], RT[:, 3 * i, :], 0.5256,
                                   RT[:, 9 + i, :], OP.mult, OP.add)
            v.scalar_tensor_tensor(XYZ[:, i, :], RT[:, 3 * i + 1, :], 1.3612,
                                   XYZ[:, i, :], OP.mult, OP.add)
            g.tensor_copy(XYZ[:, 3 + i, :], RT[:, 9 + i, :])
            v.scalar_tensor_tensor(XYZ[:, 6 + i, :], RT[:, 3 * i, :], -1.5251,
                                   RT[:, 9 + i, :], OP.mult, OP.add)

        nc.gpsimd.dma_start(out=aff_o[:, 0:9, :], in_=RT[:, 0:9, :])
        nc.scalar.dma_start(out=aff_o[:, 9:12, :], in_=RT[:, 9:12, :])
        nc.gpsimd.dma_start(out=xyz_o[0:64], in_=XYZ[0:64])
        nc.scalar.dma_start(out=xyz_o[64:128], in_=XYZ[64:128])

    nc.finalize()
    return nc


def _get_nc(use_mask: bool, use_k: bool):
    key = (use_mask, use_k)
    if key not in _NC_CACHE:
        _NC_CACHE[key] = _build(use_mask, use_k)
    return _NC_CACHE[key]


def kernel(x, affine, affine_mask, W1, b1, ln_w, ln_b, Wp, bp):
    global LAST_RESULTS
    x = np.ascontiguousarray(np.asarray(x, dtype=np.float32))
    affine = np.ascontiguousarray(np.asarray(affine, dtype=np.float32))
    mask = np.asarray(affine_mask)
    W1 = np.ascontiguousarray(np.asarray(W1, dtype=np.float32))
    b1 = np.ascontiguousarray(np.asarray(b1, dtype=np.float32))
    ln_w = np.asarray(ln_w, dtype=np.float32)
    ln_b = np.asarray(ln_b, dtype=np.float32)
    Wp = np.asarray(Wp, dtype=np.float32)
    bp = np.asarray(bp, dtype=np.float32)

    wpa9 = (ln_w[:, None] * Wp).astype(np.float32)                      # [512, 9]
    wpa = np.concatenate([wpa9, np.ones((D, 1), np.float32)], axis=1)   # [512, 10]
    ncs = (-wpa9.astype(np.float64).sum(0)).astype(np.float32)          # -colsum
    kvec = (ln_b.astype(np.float64) @ Wp.astype(np.float64)
            + bp.astype(np.float64)).astype(np.float32)                 # [9]
    use_k = bool(np.any(kvec != 0.0))
    use_mask = not bool(mask.all())

    nc = _get_nc(use_mask, use_k)

    xs = x.reshape(NCORES, TPC, D)
    affs = affine.reshape(NCORES, P, NT, 12)
    maskfs = mask.reshape(NCORES, P, NT).astype(np.float32)

    in_maps = []
    for c in range(NCORES):
        im = {
            "xt": np.ascontiguousarray(xs[c].T),
            "aff_in": affs[c],
            "w1": W1,
            "wpa": wpa,
            "b1v": b1,
            "ncs": ncs,
            "z1": _ONES_HOST,
        }
        if use_k:
            im["kv"] = kvec
            im["kv10"] = (kvec[0:3] * TRANS_SCALE).astype(np.float32)
        if use_mask:
            im["maskf"] = maskfs[c]
        in_maps.append(im)

    res = run_bass_kernel_spmd(nc, in_maps, list(range(NCORES)))
    LAST_RESULTS = res

    # [128, c, 64] planes -> [8192, c] token-major
    aff_out = np.concatenate(
        [res.results[c]["aff_o"].transpose(0, 2, 1).reshape(TPC, 12)[None]
         for c in range(NCORES)], axis=0,
    ).reshape(B, L, 12).astype(np.float32)
    xyz_out = np.concatenate(
        [res.results[c]["xyz_o"].transpose(0, 2, 1).reshape(TPC, 9)[None]
         for c in range(NCORES)], axis=0,
    ).reshape(B, L, 3, 3).astype(np.float32)
    return aff_out, xyz_out


# revision 47
# speedup vs baseline: 1.0247x; 1.0066x over previous
"""Trainium2 Bass kernel for nn_Dim6RotStructureHead.

Per token t (B*L = 65536 tokens, data-parallel over 8 cores):
  h  = x @ W1 + b1 ; g = gelu(h)
  LN stats: mu = mean(g), var = E[g^2] - mu^2
  p  = ((g - mu) * rstd * ln_w + ln_b) @ Wp + bp
     = rstd * (g @ Wpa - mu * cs) + k        (Wpa = diag(ln_w) Wp, cs = colsum(Wpa),
                                              k = ln_b @ Wp + bp)
  then Gram-Schmidt rotation + rigid compose + 3-atom placement.

Device layout: hT (d on partitions).  mm1: psum[j,tok] = W1[k,j].T @ xT[k,tok].
q = g @ [Wpa | 1] accumulates into PSUM rows 0..9; sum(g^2) runs concurrently on
PE column-group 1 (tile_position auto from psum base partition 32).
Per-token tail math is done as [128, c, 64] "plane" ops (token = p*64 + T).
"""

import contextlib

import ml_dtypes
import numpy as np

import concourse.bass as bass
import concourse.bacc as bacc
import concourse.mybir as mybir
import concourse.tile as tile
from concourse.bass_utils import run_bass_kernel_spmd

f32 = mybir.dt.float32
f32r = mybir.dt.float32r
bf16 = mybir.dt.bfloat16
AF = mybir.ActivationFunctionType
OP = mybir.AluOpType
AX = mybir.AxisListType

B, L, D = 16, 4096, 512
NCORES = 8
TPC = B * L // NCORES          # 8192 tokens per core
P = 128
NT = TPC // P                  # 64 plane columns
GT = 512                       # tokens per main-loop group
NG = TPC // GT                 # 16 groups
KC = D // P                    # 4 contraction chunks
JC = D // P                    # 4 output chunks
LN_EPS = 1e-5
TRANS_SCALE = 10.0

_NC_CACHE: dict[tuple, object] = {}
_ONES_HOST = np.ones((128, 1), np.float32)
LAST_RESULTS = None


def _bcast3(ap, c):
    """[128, 64] -> [128, c, 64] step-0 broadcast."""
    return ap[:, None, :].to_broadcast((P, c, NT))


def _part_bcast(ap):
    """1-D DRAM AP -> [128, n] partition-broadcast AP (step-0 partition dim)."""
    return bass.AP(tensor=ap.tensor, offset=ap.offset, ap=[[0, P]] + list(ap.ap))


def _build(use_mask: bool, use_k: bool):
    nc = bacc.Bacc()

    xt = nc.declare_dram_parameter("xt", [D, TPC], f32r, isOutput=False)
    aff_in = nc.declare_dram_parameter("aff_in", [P, NT, 12], f32, isOutput=False)
    w1 = nc.declare_dram_parameter("w1", [D, D], f32r, isOutput=False)
    wpa = nc.declare_dram_parameter("wpa", [D, 10], f32r, isOutput=False)
    b1v = nc.declare_dram_parameter("b1v", [D], f32, isOutput=False)
    ncs = nc.declare_dram_parameter("ncs", [9], f32, isOutput=False)
    z1 = nc.declare_dram_parameter("z1", [P, 1], f32r, isOutput=False)
    if use_k:
        kv = nc.declare_dram_parameter("kv", [9], f32, isOutput=False)
        kv10 = nc.declare_dram_parameter("kv10", [3], f32, isOutput=False)
    if use_mask:
        maskf = nc.declare_dram_parameter("maskf", [P, NT], f32, isOutput=False)

    aff_o = nc.declare_dram_parameter("aff_o", [P, 12, NT], f32, isOutput=True)
    xyz_o = nc.declare_dram_parameter("xyz_o", [P, 9, NT], f32, isOutput=True)

    v = nc.vector
    s = nc.scalar
    g = nc.gpsimd

    with tile.TileContext(nc) as tc, contextlib.ExitStack() as ctx:
        const = ctx.enter_context(tc.tile_pool(name="const", bufs=1))
        xpool = ctx.enter_context(tc.tile_pool(name="xpool", bufs=4))
        gpool = ctx.enter_context(tc.tile_pool(name="gpool", bufs=3))
        rows = ctx.enter_context(tc.tile_pool(name="rows", bufs=1))
        planes = ctx.enter_context(tc.tile_pool(name="planes", bufs=1))
        hpsum = ctx.enter_context(tc.tile_pool(name="hpsum", bufs=4, space="PSUM"))
        qpsum = ctx.enter_context(tc.tile_pool(name="qpsum", bufs=2, space="PSUM"))
        spsum = ctx.enter_context(tc.tile_pool(name="spsum", bufs=2, space="PSUM"))
        dpool = ctx.enter_context(tc.tile_pool(name="dpool", bufs=1, space="DRAM"))

        # ---- constants ----
        W1_sb = const.tile([P, KC, D], f32r)
        w1_r = w1.rearrange("(kc kp) j -> kp kc j", kp=P)
        for k in range(KC):
            nc.scalar.dma_start(out=W1_sb[:, k, :], in_=w1_r[:, k, :])
        Wpa_sb = const.tile([P, KC, 10], f32r)
        nc.scalar.dma_start(out=Wpa_sb[:], in_=wpa.rearrange("(kc kp) c -> kp kc c", kp=P))
        Z1_sb = const.tile([P, 1], f32r)
        nc.scalar.dma_start(out=Z1_sb[:], in_=z1[:])
        b1_sb = const.tile([P, JC], f32)
        nc.scalar.dma_start(out=b1_sb[:], in_=b1v.rearrange("(jc jp) -> jp jc", jp=P))
        ncs_sb = const.tile([P, 9], f32)
        nc.scalar.dma_start(out=ncs_sb[:], in_=_part_bcast(ncs[:]))
        NCS9_t = const.tile([P, 9, NT], f32)
        v.tensor_copy(NCS9_t[:], ncs_sb[:, :, None].to_broadcast((P, 9, NT)))
        if use_k:
            kv_sb = const.tile([P, 9], f32)
            nc.sync.dma_start(out=kv_sb[:], in_=_part_bcast(kv[:]))
            kv10_sb = const.tile([P, 3], f32)
            nc.sync.dma_start(out=kv10_sb[:], in_=_part_bcast(kv10[:]))
        A_t = const.tile([P, NT, 12], f32)
        nc.scalar.dma_start(out=A_t[:], in_=aff_in[:])
        if use_mask:
            M_t = const.tile([P, NT], f32)
            nc.sync.dma_start(out=M_t[:], in_=maskf[:])
        eps_sb = const.tile([P, 1], f32)
        v.memset(eps_sb[:], LN_EPS)

        PT_rows = rows.tile([11, TPC], f32)
        S2_row = rows.tile([1, TPC], f32)
        P9 = planes.tile([P, 11, NT], f32)
        ptmp = dpool.tile([11, TPC], f32)
        xt_r = xt.rearrange("(kc kp) t -> kp kc t", kp=P)

        # ---- main loop ----
        pending_s2 = None
        for gi in range(NG):
            t0 = gi * GT
            xt_sb = xpool.tile([P, KC, GT], f32r, tag="xt", name="xt_sb")
            if gi == 0:
                for k in range(KC):
                    nc.sync.dma_start(out=xt_sb[:, k, :],
                                      in_=xt_r[:, k, t0:t0 + GT])
            else:
                nc.sync.dma_start(out=xt_sb[:, 0:2, :],
                                  in_=xt_r[:, 0:2, t0:t0 + GT])
                nc.sync.dma_start(out=xt_sb[:, 2:4, :],
                                  in_=xt_r[:, 2:4, t0:t0 + GT])

            gts = []
            g2s = []
            for j in range(JC):
                ph = hpsum.tile([P, GT], f32, tag="hps", name="ph")
                for k in range(KC):
                    nc.tensor.matmul(
                        ph[:],
                        lhsT=W1_sb[:, k, j * P:(j + 1) * P],
                        rhs=xt_sb[:, k, :],
                        start=(k == 0),
                        stop=(k == KC - 1),
                    )
                gt = gpool.tile([P, GT], f32r, tag=f"gt{j}", name="gt")
                s.activation(gt[:], ph[:], AF.Gelu, bias=b1_sb[:, j:j + 1], scale=1.0)
                g2 = gpool.tile([P, GT], bf16, tag=f"g2{j}", name="g2")
                if j != 1:
                    s.activation(g2[:], gt[:], AF.Square)
                else:
                    v.tensor_tensor(g2[:], gt[:], gt[:], OP.mult)
                gts.append(gt)
                g2s.append(g2)

            # reduce the four g^2 chunks on DVE/GPSIMD; a single deferred
            # matmul (emitted under the NEXT group's mm1 shadow) does the
            # final partition-sum, saving 3 full PE token-streams per group.
            a01 = gpool.tile([P, GT], f32, tag="a01", name="a01")
            v.tensor_tensor(a01[:], g2s[0][:], g2s[1][:], OP.add)
            a23 = gpool.tile([P, GT], f32, tag="a23", name="a23")
            v.tensor_tensor(a23[:], g2s[2][:], g2s[3][:], OP.add)
            g2sum = gpool.tile([P, GT], f32r, tag="g2sum", name="g2sum")
            v.tensor_tensor(g2sum[:], a01[:], a23[:], OP.add)
            if pending_s2 is not None:
                prev_t0, prev_sum = pending_s2
                ps2 = spsum.tile([1, GT], f32, tag="sps", name="ps2")
                nc.tensor.matmul(ps2[:], lhsT=Z1_sb[:], rhs=prev_sum[:],
                                 start=True, stop=True)
                s.copy(S2_row[0:1, prev_t0:prev_t0 + GT], ps2[:])
                nc.gpsimd.dma_start(out=ptmp[10:11, prev_t0:prev_t0 + GT],
                                    in_=S2_row[0:1, prev_t0:prev_t0 + GT])
            pending_s2 = (t0, g2sum)
            pq = qpsum.tile([10, GT], f32, tag="qps", name="pq")
            for k in range(KC):
                nc.tensor.matmul(pq[:], lhsT=Wpa_sb[:, k, :], rhs=gts[k][:],
                                 start=(k == 0), stop=(k == KC - 1))
            v.tensor_copy(PT_rows[0:10, t0:t0 + GT], pq[:])

            # incremental planes import: this group's 512 tokens are
            # partitions 8*gi .. 8*gi+7 of the plane layout (token = p*64+T)
            nc.gpsimd.dma_start(out=ptmp[0:10, t0:t0 + GT],
                                in_=PT_rows[0:10, t0:t0 + GT])
            nc.gpsimd.dma_start(out=ptmp[10:11, t0:t0 + GT],
                                in_=S2_row[0:1, t0:t0 + GT])
            nc.gpsimd.dma_start(
                out=P9[8 * gi:8 * gi + 8, 0:10, :],
                in_=ptmp[0:10, t0:t0 + GT].rearrange("c (p t) -> p c t", p=8))
            if gi >= 1:
                p0 = (gi - 1) * GT
                nc.gpsimd.dma_start(
                    out=P9[8 * (gi - 1):8 * (gi - 1) + 8, 10:11, :],
                    in_=ptmp[10:11, p0:p0 + GT].rearrange("c (p t) -> p c t", p=8))

        prev_t0, prev_sum = pending_s2
        ps2f = spsum.tile([1, GT], f32, tag="sps", name="ps2f")
        nc.tensor.matmul(ps2f[:], lhsT=Z1_sb[:], rhs=prev_sum[:],
                         start=True, stop=True)
        s.copy(S2_row[0:1, prev_t0:prev_t0 + GT], ps2f[:])
        nc.sync.dma_start(out=ptmp[10:11, prev_t0:prev_t0 + GT],
                          in_=S2_row[0:1, prev_t0:prev_t0 + GT])
        nc.sync.dma_start(
            out=P9[8 * (NG - 1):, 10:11, :],
            in_=ptmp[10:11, prev_t0:prev_t0 + GT].rearrange(
                "c (p t) -> p c t", p=8))

        def pt(shape, name):
            return planes.tile(shape, f32, tag=name, name=name)

        S1 = P9[:, 9, :]
        S2 = P9[:, 10, :]
        mu = pt([P, NT], "mu")
        v.tensor_scalar_mul(mu[:], S1, 1.0 / D)
        mu2 = pt([P, NT], "mu2")
        v.tensor_tensor(mu2[:], mu[:], mu[:], OP.mult)
        e2t = pt([P, NT], "e2t")
        v.tensor_scalar_mul(e2t[:], S2, 1.0 / D)
        varr = pt([P, NT], "varr")
        v.scalar_tensor_tensor(varr[:], mu2[:], -1.0, e2t[:], OP.mult, OP.add)
        A_ = pt([P, NT], "A_")
        s.activation(A_[:], varr[:], AF.Sqrt, bias=eps_sb[:, 0:1])
        rstd = pt([P, NT], "rstd")
        v.reciprocal(rstd[:], A_[:])
        rstd10 = pt([P, NT], "rstd10")
        v.tensor_scalar_mul(rstd10[:], rstd[:], TRANS_SCALE)

        # U_c = q_c - mu*cs_c   (ncs holds -cs)
        U = planes.tile([P, 9, NT], f32)
        MC = pt([P, 9, NT], "MC")
        v.tensor_tensor(MC[:], NCS9_t[:], _bcast3(mu, 9), OP.mult)
        v.tensor_tensor(U[:], P9[:, 0:9, :], MC[:], OP.add)

        TR = pt([P, 3, NT], "TR")
        v.tensor_tensor(TR[:], U[:, 0:3, :], _bcast3(rstd10, 3), OP.mult)
        P36 = pt([P, 6, NT], "P36")
        v.tensor_tensor(P36[:], U[:, 3:9, :], _bcast3(rstd, 6), OP.mult)
        if use_k:
            for c in range(3):
                v.tensor_scalar_add(TR[:, c, :], TR[:, c, :], kv10_sb[:, c:c + 1])
            for c in range(6):
                v.tensor_scalar_add(P36[:, c, :], P36[:, c, :], kv_sb[:, 3 + c:4 + c])

        SQ6 = pt([P, 6, NT], "SQ6")
        v.tensor_tensor(SQ6[:], P36[:], P36[:], OP.mult)
        M2 = pt([P, 2, NT], "M2")
        v.tensor_reduce(M2[:, 0, :], SQ6[:, 0:3, :].rearrange("p c t -> p t c"),
                        axis=AX.X, op=OP.add)
        v.tensor_reduce(M2[:, 1, :], SQ6[:, 3:6, :].rearrange("p c t -> p t c"),
                        axis=AX.X, op=OP.add)
        NXY = pt([P, 2, NT], "NXY")
        s.activation(NXY[:], M2[:], AF.Sqrt)

        DEN = pt([P, 2, NT], "DEN")
        v.tensor_scalar_add(DEN[:], NXY[:], 1e-5)
        CXY = pt([P, 2, NT], "CXY")
        v.reciprocal(CXY[:], DEN[:])
        cx = CXY[:, 0, :]
        cy = CXY[:, 1, :]

        VX = pt([P, 3, NT], "VX")
        v.tensor_tensor(VX[:], P36[:, 0:3, :], _bcast3(CXY[:, 0, :], 3), OP.mult)
        VY = pt([P, 3, NT], "VY")
        v.tensor_tensor(VY[:], P36[:, 3:6, :], _bcast3(CXY[:, 1, :], 3), OP.mult)

        # e1 = -vec_x (unit to ~1e-6; reference's renorm is absmax-negligible)
        EALL = planes.tile([P, 3, 3, NT], f32)
        v.tensor_scalar_mul(EALL[:, 0, :, :], VX[:], -1.0)
        XY = VY

        DP = pt([P, NT, 3], "DP")
        v.tensor_tensor(DP[:], XY[:].rearrange("p c t -> p t c"),
                        EALL[:, 0, :, :].rearrange("p c t -> p t c"), OP.mult)
        dd = pt([P, NT], "dd")
        v.tensor_reduce(dd[:], DP[:], axis=AX.X, op=OP.add)
        T6 = pt([P, 3, NT], "T6")
        v.tensor_tensor(T6[:], EALL[:, 0, :, :], _bcast3(dd, 3), OP.mult)
        U2 = pt([P, 3, NT], "U2")
        v.tensor_tensor(U2[:], XY[:], T6[:], OP.subtract)

        SQ3 = pt([P, 3, NT], "SQ3")
        v.tensor_tensor(SQ3[:], U2[:], U2[:], OP.mult)
        m2u = pt([P, NT], "m2u")
        v.tensor_reduce(m2u[:], SQ3[:].rearrange("p c t -> p t c"),
                        axis=AX.X, op=OP.add)
        su = pt([P, NT], "su")
        s.activation(su[:], m2u[:], AF.Sqrt)
        v.tensor_scalar_add(su[:], su[:], 1e-10)
        r3 = pt([P, NT], "r3")
        v.reciprocal(r3[:], su[:])
        v.tensor_tensor(EALL[:, 1, :, :], U2[:], _bcast3(r3, 3), OP.mult)

        # e3 = e1 x e2  (gpsimd)
        ca = pt([P, NT], "ca")
        cb = pt([P, NT], "cb")
        E1 = EALL[:, 0, :, :]
        E2 = EALL[:, 1, :, :]
        for i in range(3):
            i1, i2 = (i + 1) % 3, (i + 2) % 3
            v.tensor_tensor(ca[:], E1[:, i1, :], E2[:, i2, :], OP.mult)
            v.tensor_tensor(cb[:], E1[:, i2, :], E2[:, i1, :], OP.mult)
            v.tensor_tensor(EALL[:, 2, i, :], ca[:], cb[:], OP.subtract)

        # mask: Ru -> m*(Ru - I) + I ; tu -> m*trans
        if use_mask:
            mtmp = pt([P, NT], "mtmp")
            for j in range(3):
                for k in range(3):
                    if j == k:
                        v.tensor_scalar_add(mtmp[:], EALL[:, j, k, :], -1.0)
                        v.tensor_tensor(mtmp[:], mtmp[:], M_t[:], OP.mult)
                        v.tensor_scalar_add(EALL[:, j, k, :], mtmp[:], 1.0)
                    else:
                        v.tensor_tensor(EALL[:, j, k, :], EALL[:, j, k, :],
                                        M_t[:], OP.mult)
            TU = pt([P, 3, NT], "TU")
            g.tensor_tensor(TU[:], TR[:], _bcast3(M_t[:], 3), OP.mult)
        else:
            TU = TR

        # R[i][j] = dot(Rp_row_i, e_j) ; t = Rp @ tu + tp
        RT = planes.tile([P, 12, NT], f32)
        for i in range(3):
            for j in range(3):
                DPR = pt([P, NT, 3], f"DPR{(i + j) % 2}")
                v.tensor_tensor(DPR[:], A_t[:, :, 3 * i:3 * i + 3],
                                EALL[:, j, :, :].rearrange("p k t -> p t k"),
                                OP.mult)
                v.tensor_reduce(RT[:, 3 * i + j, :], DPR[:], axis=AX.X, op=OP.add)
        for i in range(3):
            DPT = pt([P, NT, 3], f"DPT{i % 2}")
            v.tensor_tensor(DPT[:], A_t[:, :, 3 * i:3 * i + 3],
                            TU[:].rearrange("p k t -> p t k"), OP.mult)
            tacc = pt([P, NT], f"tacc{i}")
            v.tensor_reduce(tacc[:], DPT[:], axis=AX.X, op=OP.add)
            v.tensor_tensor(RT[:, 9 + i, :], tacc[:], A_t[:, :, 9 + i], OP.add)

        # pred_xyz
        XYZ = planes.tile([P, 9, NT], f32)
        for i in range(3):
            v.scalar_tensor_tensor(XYZ[:, i, :], RT[:, 3 * i, :], 0.5256,
                                   RT[:, 9 + i, :], OP.mult, OP.add)
            v.scalar_tensor_tensor(XYZ[:, i, :], RT[:, 3 * i + 1, :], 1.3612,
                                   XYZ[:, i, :], OP.mult, OP.add)
            g.tensor_copy(XYZ[:, 3 + i, :], RT[:, 9 + i, :])
            v.scalar_tensor_tensor(XYZ[:, 6 + i, :], RT[:, 3 * i, :], -1.5251,
                                   RT[:, 9 + i, :], OP.mult, OP.add)

        nc.gpsimd.dma_start(out=aff_o[:, 0:9, :], in_=RT[:, 0:9, :])
        nc.scalar.dma_start(out=aff_o[:, 9:12, :], in_=RT[:, 9:12, :])
        nc.gpsimd.dma_start(out=xyz_o[0:64], in_=XYZ[0:64])
        nc.scalar.dma_start(out=xyz_o[64:128], in_=XYZ[64:128])

    nc.finalize()
    return nc


def _get_nc(use_mask: bool, use_k: bool):
    key = (use_mask, use_k)
    if key not in _NC_CACHE:
        _NC_CACHE[key] = _build(use_mask, use_k)
    return _NC_CACHE[key]


def kernel(x, affine, affine_mask, W1, b1, ln_w, ln_b, Wp, bp):
    global LAST_RESULTS
    x = np.ascontiguousarray(np.asarray(x, dtype=np.float32))
    affine = np.ascontiguousarray(np.asarray(affine, dtype=np.float32))
    mask = np.asarray(affine_mask)
    W1 = np.ascontiguousarray(np.asarray(W1, dtype=np.float32))
    b1 = np.ascontiguousarray(np.asarray(b1, dtype=np.float32))
    ln_w = np.asarray(ln_w, dtype=np.float32)
    ln_b = np.asarray(ln_b, dtype=np.float32)
    Wp = np.asarray(Wp, dtype=np.float32)
    bp = np.asarray(bp, dtype=np.float32)

    wpa9 = (ln_w[:, None] * Wp).astype(np.float32)                      # [512, 9]
    wpa = np.concatenate([wpa9, np.ones((D, 1), np.float32)], axis=1)   # [512, 10]
    ncs = (-wpa9.astype(np.float64).sum(0)).astype(np.float32)          # -colsum
    kvec = (ln_b.astype(np.float64) @ Wp.astype(np.float64)
            + bp.astype(np.float64)).astype(np.float32)                 # [9]
    use_k = bool(np.any(kvec != 0.0))
    use_mask = not bool(mask.all())

    nc = _get_nc(use_mask, use_k)

    xs = x.reshape(NCORES, TPC, D)
    affs = affine.reshape(NCORES, P, NT, 12)
    maskfs = mask.reshape(NCORES, P, NT).astype(np.float32)

    in_maps = []
    for c in range(NCORES):
        im = {
            "xt": np.ascontiguousarray(xs[c].T),
            "aff_in": affs[c],
            "w1": W1,
            "wpa": wpa,
            "b1v": b1,
            "ncs": ncs,
            "z1": _ONES_HOST,
        }
        if use_k:
            im["kv"] = kvec
            im["kv10"] = (kvec[0:3] * TRANS_SCALE).astype(np.float32)
        if use_mask:
            im["maskf"] = maskfs[c]
        in_maps.append(im)

    res = run_bass_kernel_spmd(nc, in_maps, list(range(NCORES)))
    LAST_RESULTS = res

    # [128, c, 64] planes -> [8192, c] token-major
    aff_out = np.concatenate(
        [res.results[c]["aff_o"].transpose(0, 2, 1).reshape(TPC, 12)[None]
         for c in range(NCORES)], axis=0,
    ).reshape(B, L, 12).astype(np.float32)
    xyz_out = np.concatenate(
        [res.results[c]["xyz_o"].transpose(0, 2, 1).reshape(TPC, 9)[None]
         for c in range(NCORES)], axis=0,
    ).reshape(B, L, 3, 3).astype(np.float32)
    return aff_out, xyz_out
